# revision 14
# baseline (speedup 1.0000x reference)
"""Multi-head sigmoid self-attention on 8 Trainium2 NeuronCores.

Sharding: pure data parallel - batch (8) split one element per core.

Per-core design (engine-balance driven):
  * The Scalar (ACT) engine is the critical resource: 12.6M sigmoids
    ~= 82 us of pure streaming at 1 elem/cycle/lane @1.2GHz.  All
    scheduling exists to keep ACT 100% busy from ~5 us to the end.
  * Scores stream through a 6-bank PSUM ring (1 bank = one 128x512
    score tile for one head-half of one key tile).  The ACT consumes
    the ring in FD=1536 calls (3 slots each, 64 calls total) to
    amortize the ~280-cycle per-call overhead.
  * Slot-granular software pipeline: per 3-slot group the emission is
    [3 score MMs][1 ACT][3 attn MMs (lagged 4 groups)][filler units].
    Fillers (q/k/v/o projections) are deadline-scheduled so the PE
    never head-of-line blocks the score stream.
  * q/k projections run in fp8 (e4m3) with DoubleRow perf mode (2
    MACs/cell/cycle).  Host pre-scales x by SX and Wq/Wk by SW to
    center values in e4m3 range; the combined 1/(SX*SW)^2 factor is
    folded into the sigmoid's free affine scale - zero extra cost.
    v/o stay bf16 (their error reaches the output unsuppressed).
  * PE warm-up: dummy matmuls at t=0 get the HAM clock-gate to 2.4GHz
    before the first real projection; a dummy sigmoid at t=0 pulls
    the ACT table load (~2.7us) into the DMA wait.
  * Tail: the last chunk's output projection is split ko 0-2
    (precomputed into SBUF) + ko 3-5 (tail group on freed ring banks)
    so only ~5 us of work follows the final sigmoid.
"""

import os
import sys

import numpy as np

if "/opt/trn_rl_repo" not in sys.path:
    sys.path.insert(0, "/opt/trn_rl_repo")

P = 128
F = 768
N = 1024
H = 12
HD = 64
KO = 6        # 128-feature stripes (bf16 path)
KO8 = 3       # 256-feature DoubleRow stripes (fp8 path)
NT = 8        # token tiles
CH = 2        # query chunks
CW = 512      # chunk width
HP = 6        # head pairs
FW = 384      # o/v projection feature chunk
RING = 6      # psum score ring slots (1 bank each)
GRP = 3       # ring slots per ACT call (FD = 1536)
SLOTS = 192   # 12 pairs x 16 (8 kt x 2 head-halves)
GROUPS = SLOTS // GRP
LAG = 4       # attn lag behind ACT, in groups

QK_MODE = os.environ.get("QK_MODE", "fp8qk")  # fp8qk | fp8k | bf16
SX = 2.0      # x pre-scale for fp8
SW = 16.0     # Wq/Wk pre-scale for fp8
SCALE = 1.0 / float(np.sqrt(np.float32(F)))

# ch-major pair order: all of chunk 0's head pairs, then chunk 1's.
PAIRS = [(ch, hp) for ch in range(CH) for hp in range(HP)]

_CACHE = {}
LAST_EXEC_NS = None


def _build():
    import concourse.mybir as mybir
    import concourse.tile as tile
    from concourse import bacc

    f32 = mybir.dt.float32
    bf16 = mybir.dt.bfloat16
    fp8 = mybir.dt.float8e4
    ADD = mybir.AluOpType.add
    SIG = mybir.ActivationFunctionType.Sigmoid
    DR = mybir.MatmulPerfMode.DoubleRow

    q_fp8 = QK_MODE == "fp8qk"
    k_fp8 = QK_MODE in ("fp8qk", "fp8k")
    any_fp8 = q_fp8 or k_fp8
    cq = SX * SW if q_fp8 else 1.0
    ck = SX * SW if k_fp8 else 1.0
    scale_act = SCALE / (cq * ck)

    nc = bacc.Bacc("TRN2", target_bir_lowering=False, debug=False)

    xT_d = nc.dram_tensor("xT", [P, KO, N], bf16, kind="ExternalInput").ap()
    if any_fp8:
        x8_d = nc.dram_tensor("x8", [P, KO, N], fp8, kind="ExternalInput").ap()
    if q_fp8:
        wq_d = nc.dram_tensor("wqT", [P, KO, F], fp8, kind="ExternalInput").ap()
    else:
        wq_d = nc.dram_tensor("wqT", [P, KO, F], bf16, kind="ExternalInput").ap()
    if k_fp8:
        wk_d = nc.dram_tensor("wkT", [P, KO, F], fp8, kind="ExternalInput").ap()
    else:
        wk_d = nc.dram_tensor("wkT", [P, KO, F], bf16, kind="ExternalInput").ap()
    wv_d = nc.dram_tensor("wvT", [P, KO, F], bf16, kind="ExternalInput").ap()
    wo_d = nc.dram_tensor("woT", [P, KO, F], bf16, kind="ExternalInput").ap()
    bq_d = nc.dram_tensor("bqs", [P, KO], f32, kind="ExternalInput").ap()
    bk_d = nc.dram_tensor("bks", [P, KO], f32, kind="ExternalInput").ap()
    bv_d = nc.dram_tensor("bvr", [P, F], f32, kind="ExternalInput").ap()
    bo_d = nc.dram_tensor("bor", [P, F], f32, kind="ExternalInput").ap()
    bi_d = nc.dram_tensor("bir", [P, 1], f32, kind="ExternalInput").ap()
    o_d = nc.dram_tensor("o", [N, F], f32, kind="ExternalOutput").ap()
    debug = bool(os.environ.get("KERNEL_DEBUG"))
    if debug:
        dbg_q = nc.dram_tensor("dbg_q", [P, KO, N], bf16,
                               kind="ExternalOutput").ap()
        dbg_k = nc.dram_tensor("dbg_k", [P, KO, N], bf16,
                               kind="ExternalOutput").ap()
        dbg_v = nc.dram_tensor("dbg_v", [P, NT, F], bf16,
                               kind="ExternalOutput").ap()
        dbg_a = nc.dram_tensor("dbg_a", [P, KO, N], bf16,
                               kind="ExternalOutput").ap()
        dbg_sc = nc.dram_tensor("dbg_sc", [P, GRP, CW], f32,
                                kind="ExternalOutput").ap()
        dbg_st = nc.dram_tensor("dbg_st", [P, GRP, CW], bf16,
                                kind="ExternalOutput").ap()

    with tile.TileContext(nc) as tc:
        with (
            tc.tile_pool(name="sb", bufs=1) as sb,
            tc.tile_pool(name="ps", bufs=1, space="PSUM") as psp,
        ):
            # ---- persistent SBUF -------------------------------------
            xT = sb.tile([P, KO, N], bf16, tag="xT")
            if any_fp8:
                x8 = sb.tile([P, KO, N], fp8, tag="x8")
            wq = sb.tile([P, KO, F], fp8 if q_fp8 else bf16, tag="wq")
            wk = sb.tile([P, KO, F], fp8 if k_fp8 else bf16, tag="wk")
            wv = sb.tile([P, KO, F], bf16, tag="wv")
            wo = sb.tile([P, KO, F], bf16, tag="wo")
            qT = sb.tile([P, KO, N], bf16, tag="qT")
            kT = sb.tile([P, KO, N], bf16, tag="kT")
            v_sb = sb.tile([P, NT, F], bf16, tag="v")
            attnT = sb.tile([P, KO, N], bf16, tag="attnT")
            o_part = sb.tile([P, 4, F], f32, tag="opart")
            bqs = sb.tile([P, KO], f32, tag="bqs")
            bks = sb.tile([P, KO], f32, tag="bks")
            bvr = sb.tile([P, F], f32, tag="bvr")
            bor = sb.tile([P, F], f32, tag="bor")
            bir = sb.tile([P, 1], f32, tag="bir")
            warm_w = sb.tile([P, P], bf16, tag="warmw")
            warm_x = sb.tile([P, 256], bf16, tag="warmx")
            dum_i = sb.tile([P, 8], f32, tag="dumi")
            dum_o = sb.tile([P, 8], bf16, tag="dumo")

            # ---- t=0 priming -----------------------------------------
            # memsets first so the PE warm-up isn't queued behind DMAs
            nc.gpsimd.memset(dum_i[:], 0.0)
            nc.gpsimd.memset(warm_w[:], 0.0)
            nc.gpsimd.memset(warm_x[:], 0.0)
            # dummy sigmoid pulls the ACT table load into the DMA wait
            nc.scalar.activation(dum_o[:], dum_i[:], SIG)
            nc.sync.dma_start(bir[:], bi_d)
            nc.sync.dma_start(bks[:], bk_d)
            nc.sync.dma_start(bqs[:], bq_d)
            if any_fp8:
                for ko in range(KO):
                    nc.sync.dma_start(x8[:, ko, :], x8_d[:, ko, :])
            else:
                for ko in range(KO):
                    nc.sync.dma_start(xT[:, ko, :], xT_d[:, ko, :])
            for ko in range(KO):
                nc.gpsimd.dma_start(wk[:, ko, :], wk_d[:, ko, :])
            for ko in range(KO):
                nc.gpsimd.dma_start(wq[:, ko, :], wq_d[:, ko, :])
            # PE warm-up: ~16 cold matmuls ~= 3.4us of activity gets the
            # HAM to open the clock gate before real projections arrive.
            warm_ps = psp.tile([P, CW], f32, tag="pp", bufs=1, name="pp")
            for _ in range(16):
                nc.tensor.matmul(warm_ps[:, 0:256], warm_w[:], warm_x[:],
                                 start=True, stop=True)
            if any_fp8:
                for ko in range(KO):
                    nc.sync.dma_start(xT[:, ko, :], xT_d[:, ko, :])
            for ko in range(KO):
                nc.gpsimd.dma_start(wv[:, ko, :], wv_d[:, ko, :])
            nc.sync.dma_start(bvr[:], bv_d)
            for ko in range(KO):
                nc.sync.dma_start(wo[:, ko, :], wo_d[:, ko, :])
            nc.sync.dma_start(bor[:], bo_d)

            # ---- projection / output units ---------------------------
            def unit_qk(proj, mo, ch):
                """q or k projection: stripe mo, chunk ch -> qT/kT."""
                use8 = k_fp8 if proj == "k" else q_fp8
                w_sb = wk if proj == "k" else wq
                bst = bks if proj == "k" else bqs
                dst = kT if proj == "k" else qT
                csl = slice(ch * CW, (ch + 1) * CW)
                ps = psp.tile([P, CW], f32, tag="pp", bufs=1, name="pp")
                if use8:
                    for k3 in range(KO8):
                        nc.tensor.matmul(
                            ps[:],
                            w_sb[:, 2 * k3:2 * k3 + 2, mo * P:(mo + 1) * P],
                            x8[:, 2 * k3:2 * k3 + 2, csl],
                            start=(k3 == 0), stop=(k3 == KO8 - 1),
                            perf_mode=DR,
                        )
                else:
                    for ko in range(KO):
                        nc.tensor.matmul(
                            ps[:],
                            w_sb[:, ko, mo * P:(mo + 1) * P],
                            xT[:, ko, csl],
                            start=(ko == 0), stop=(ko == KO - 1),
                        )
                nc.vector.tensor_tensor(
                    dst[:, mo, csl], ps[:],
                    bst[:, mo:mo + 1].to_broadcast([P, CW]), ADD,
                )

            def unit_v(t, j):
                """v projection: token tile t, feature chunk j."""
                ps = psp.tile([P, CW], f32, tag="pp", bufs=1, name="pp")
                psv = ps[:, 0:FW]
                for ko in range(KO):
                    nc.tensor.matmul(
                        psv,
                        xT[:, ko, t * P:(t + 1) * P],
                        wv[:, ko, j * FW:(j + 1) * FW],
                        start=(ko == 0), stop=(ko == KO - 1),
                    )
                nc.vector.tensor_tensor(
                    v_sb[:, t, j * FW:(j + 1) * FW], psv,
                    bvr[:, j * FW:(j + 1) * FW], ADD,
                )

            osb_tiles = {}

            def get_osb(tt):
                if tt not in osb_tiles:
                    osb_tiles[tt] = sb.tile([P, F], f32, tag="osb", bufs=3,
                                            name="osb")
                return osb_tiles[tt]

            def unit_o_full(tg, j):
                """chunk-0 output projection: token tile tg, half j."""
                ps = psp.tile([P, CW], f32, tag="pp", bufs=1, name="pp")
                pso = ps[:, 0:FW]
                for ko in range(KO):
                    nc.tensor.matmul(
                        pso,
                        attnT[:, ko, tg * P:(tg + 1) * P],
                        wo[:, ko, j * FW:(j + 1) * FW],
                        start=(ko == 0), stop=(ko == KO - 1),
                    )
                osb = get_osb(tg)
                nc.vector.tensor_tensor(
                    osb[:, j * FW:(j + 1) * FW], pso,
                    bor[:, j * FW:(j + 1) * FW], ADD,
                )
                if j == 1:
                    nc.sync.dma_start(o_d[tg * P:(tg + 1) * P, :], osb[:])
                    del osb_tiles[tg]

            def unit_o_part(tg, j):
                """chunk-1 partial o-proj (ko 0-2 + bias) -> o_part."""
                tt = 4 + tg
                ps = psp.tile([P, CW], f32, tag="pp", bufs=1, name="pp")
                pso = ps[:, 0:FW]
                for ko in range(3):
                    nc.tensor.matmul(
                        pso,
                        attnT[:, ko, tt * P:(tt + 1) * P],
                        wo[:, ko, j * FW:(j + 1) * FW],
                        start=(ko == 0), stop=(ko == 2),
                    )
                nc.vector.tensor_tensor(
                    o_part[:, tg, j * FW:(j + 1) * FW], pso,
                    bor[:, j * FW:(j + 1) * FW], ADD,
                )

            def unit_o_tail(tg):
                """chunk-1 final o-proj: ko 3-5 on freed score banks."""
                tt = 4 + tg
                ot = psp.tile([P, GRP, CW], f32, tag="sc", bufs=2, name="sc")
                osb = get_osb(tt)
                for j in range(2):
                    pso = ot[:, j, 0:FW]
                    for ko in (3, 4, 5):
                        nc.tensor.matmul(
                            pso,
                            attnT[:, ko, tt * P:(tt + 1) * P],
                            wo[:, ko, j * FW:(j + 1) * FW],
                            start=(ko == 3), stop=(ko == 5),
                        )
                    nc.vector.tensor_tensor(
                        osb[:, j * FW:(j + 1) * FW], pso,
                        o_part[:, tg, j * FW:(j + 1) * FW], ADD,
                    )
                nc.sync.dma_start(o_d[tt * P:(tt + 1) * P, :], osb[:])
                del osb_tiles[tt]

            # ---- score / sigmoid / attention slots -------------------
            def slot_of(S):
                pair, s = divmod(S, 16)
                ch, hp = PAIRS[pair]
                kt, hd = divmod(s, 2)
                return pair, ch, hp, kt, hd

            st_tiles = {}

            def emit_score_group(g):
                sc = psp.tile([P, GRP, CW], f32, tag="sc", bufs=2, name="sc")
                for off in range(GRP):
                    S = g * GRP + off
                    pair, ch, hp, kt, hd = slot_of(S)
                    nc.tensor.matmul(
                        sc[:, off, :],
                        kT[64 * hd:64 * (hd + 1), hp, kt * P:(kt + 1) * P],
                        qT[64 * hd:64 * (hd + 1), hp, ch * CW:(ch + 1) * CW],
                        start=True, stop=True,
                    )
                return sc

            def emit_act(g, sc):
                st = sb.tile([P, GRP, CW], bf16, tag="st", bufs=8, name="st")
                nc.scalar.activation(st[:], sc[:], SIG,
                                     bias=bir[:, 0:1], scale=scale_act)
                st_tiles[g] = st

            def emit_attn_pair(pair):
                """contiguous 16-MM attention burst for one pair."""
                ch, hp = PAIRS[pair]
                at = psp.tile([P, CW], f32, tag="at", bufs=1, name="at")
                for s in range(16):
                    S = 16 * pair + s
                    kt, hd = divmod(s, 2)
                    g, off = divmod(S, GRP)
                    nc.tensor.matmul(
                        at[64 * hd:64 * (hd + 1), :],
                        v_sb[:, kt, hp * P + 64 * hd: hp * P + 64 * (hd + 1)],
                        st_tiles[g][:, off, :],
                        start=(kt == 0), stop=(kt == NT - 1),
                    )
                nc.vector.tensor_copy(attnT[:, hp, ch * CW:(ch + 1) * CW],
                                      at[:])

            # ---- filler schedule (due_group, emit_fn) ----------------
            fillers = []

            def add(due, fn, *args):
                fillers.append((due, fn, args))

            for s in range(1, HP):  # k/q chunk-0 stripes 1-5
                due = max(0, ((16 * s) + 2) // 3 - 3)
                add(due, unit_qk, "k", s, 0)
                add(due, unit_qk, "k", s, 1)
                add(due, unit_qk, "q", s, 0)
            # v must be fully EMITTED before the attn burst that reads it
            # (a read emitted before its write gets no dependency):
            # chunk 0 (heads 0-5) before burst(pair0) at group 5, chunk 1
            # (heads 6-11) before burst(pair3) at group 21.
            for t in range(NT):
                add(t // 2, unit_v, t, 0)
            for t in range(NT):
                add(10 + t // 2, unit_v, t, 1)
            for s in range(HP):     # q chunk-1 stripes
                add(27 + 5 * s, unit_qk, "q", s, 1)
            for u in range(8):      # chunk-0 o-proj (needs all ch0 attnT)
                add(36 + u, unit_o_full, u // 2, u % 2)
            for u in range(8):      # chunk-1 partial o-proj (ko 0-2)
                add(52 + u, unit_o_part, u // 2, u % 2)
            fillers.sort(key=lambda e: e[0])

            # ---- pre-loop: stripe-0 projections ----------------------
            unit_qk("k", 0, 0)
            unit_qk("k", 0, 1)
            unit_qk("q", 0, 0)

            if debug:
                sc_snap = sb.tile([P, GRP, CW], f32, tag="scsnap")
                st_snap = sb.tile([P, GRP, CW], bf16, tag="stsnap")

            # pair p's last score slot lives in group (16p+15)//3
            burst_at = {(16 * p + 15) // GRP: p for p in range(len(PAIRS))}

            # ---- main slot loop --------------------------------------
            fi = 0
            for g in range(GROUPS):
                sc = emit_score_group(g)
                if debug and g == 0:
                    nc.vector.tensor_copy(sc_snap[:], sc[:])
                emit_act(g, sc)
                if debug and g == 0:
                    nc.vector.tensor_copy(st_snap[:], st_tiles[0][:])
                if g in burst_at:
                    emit_attn_pair(burst_at[g])
                while fi < len(fillers) and fillers[fi][0] <= g:
                    _, fn, args = fillers[fi]
                    fn(*args)
                    fi += 1

            # ---- drain -----------------------------------------------
            while fi < len(fillers):
                _, fn, args = fillers[fi]
                fn(*args)
                fi += 1
            for tg in range(4):
                unit_o_tail(tg)
            if debug:
                nc.sync.dma_start(dbg_q, qT[:])
                nc.sync.dma_start(dbg_k, kT[:])
                nc.sync.dma_start(dbg_v, v_sb[:])
                nc.sync.dma_start(dbg_a, attnT[:])
                nc.sync.dma_start(dbg_sc, sc_snap[:])
                nc.sync.dma_start(dbg_st, st_snap[:])

    nc.compile()
    return nc


def _bf16(a):
    import ml_dtypes
    return np.ascontiguousarray(a).astype(ml_dtypes.bfloat16)


def _fp8(a):
    import ml_dtypes
    return np.ascontiguousarray(a).astype(ml_dtypes.float8_e4m3)


def _stripe(W):
    """[F_in, F_out] -> [P, KO, F_out] feature-major stripes."""
    return np.ascontiguousarray(
        np.asarray(W, np.float32).reshape(KO, P, F).transpose(1, 0, 2))


def kernel(x, bias, Wq, bq, Wk, bk, Wv, bv, Wo, bo):
    global LAST_EXEC_NS
    from concourse import bass_utils

    q_fp8 = QK_MODE == "fp8qk"
    k_fp8 = QK_MODE in ("fp8qk", "fp8k")
    any_fp8 = q_fp8 or k_fp8
    cq = SX * SW if q_fp8 else 1.0
    ck = SX * SW if k_fp8 else 1.0

    if "nc" not in _CACHE:
        _CACHE["nc"] = _build()
    nc = _CACHE["nc"]

    x = np.asarray(x, dtype=np.float32)
    Wq = np.asarray(Wq, np.float32).T
    Wk = np.asarray(Wk, np.float32).T
    shared = {
        "wqT": _fp8(_stripe(Wq * SW)) if q_fp8 else _bf16(_stripe(Wq)),
        "wkT": _fp8(_stripe(Wk * SW)) if k_fp8 else _bf16(_stripe(Wk)),
        "wvT": _bf16(_stripe(np.asarray(Wv, np.float32).T)),
        "woT": _bf16(_stripe(np.asarray(Wo, np.float32).T)),
        "bqs": np.ascontiguousarray(
            (np.asarray(bq, np.float32) * cq).reshape(KO, P).T),
        "bks": np.ascontiguousarray(
            (np.asarray(bk, np.float32) * ck).reshape(KO, P).T),
        "bvr": np.ascontiguousarray(
            np.broadcast_to(np.asarray(bv, np.float32), (P, F))),
        "bor": np.ascontiguousarray(
            np.broadcast_to(np.asarray(bo, np.float32), (P, F))),
        "bir": np.full((P, 1), np.float32(np.asarray(bias)),
                       dtype=np.float32),
    }
    in_maps = []
    for b in range(x.shape[0]):
        m = dict(shared)
        xb = np.ascontiguousarray(x[b].T.reshape(KO, P, N).transpose(1, 0, 2))
        m["xT"] = _bf16(xb)
        if any_fp8:
            m["x8"] = _fp8(xb * SX)
        in_maps.append(m)

    trace = bool(os.environ.get("KERNEL_TRACE"))
    tdir = os.environ.get("KERNEL_TRACE_DIR") or None
    if trace:
        try:
            import ntff_hook
            ntff_hook.install()
        except Exception:
            trace = False

    res = bass_utils.run_bass_kernel_spmd(
        nc, in_maps, core_ids=list(range(len(in_maps))), trace=trace,
        tmpdir=(tdir if trace else None))
    LAST_EXEC_NS = res.exec_time_ns
    globals()["LAST_RES"] = res
    return np.stack([r["o"] for r in res.results]).astype(np.float32)


# revision 21
# speedup vs baseline: 1.0171x; 1.0171x over previous
"""Multi-head sigmoid self-attention on 8 Trainium2 NeuronCores.

Sharding: pure data parallel - batch (8) split one element per core.

Per-core design (engine-balance driven):
  * The Scalar (ACT) engine is the critical resource: 12.6M sigmoids
    ~= 82 us of pure streaming at 1 elem/cycle/lane @1.2GHz.  All
    scheduling exists to keep ACT 100% busy from ~5 us to the end.
  * Scores stream through a 6-bank PSUM ring (1 bank = one 128x512
    score tile for one head-half of one key tile).  The ACT consumes
    the ring in FD=1536 calls (3 slots each, 64 calls total) to
    amortize the ~280-cycle per-call overhead.
  * Slot-granular software pipeline: per 3-slot group the emission is
    [3 score MMs][1 ACT][3 attn MMs (lagged 4 groups)][filler units].
    Fillers (q/k/v/o projections) are deadline-scheduled so the PE
    never head-of-line blocks the score stream.
  * q/k projections run in fp8 (e4m3) with DoubleRow perf mode (2
    MACs/cell/cycle).  Host pre-scales x by SX and Wq/Wk by SW to
    center values in e4m3 range; the combined 1/(SX*SW)^2 factor is
    folded into the sigmoid's free affine scale - zero extra cost.
    v/o stay bf16 (their error reaches the output unsuppressed).
  * PE warm-up: dummy matmuls at t=0 get the HAM clock-gate to 2.4GHz
    before the first real projection; a dummy sigmoid at t=0 pulls
    the ACT table load (~2.7us) into the DMA wait.
  * Tail: the last chunk's output projection is split ko 0-2
    (precomputed into SBUF) + ko 3-5 (tail group on freed ring banks)
    so only ~5 us of work follows the final sigmoid.
"""

import os
import sys

import numpy as np

if "/opt/trn_rl_repo" not in sys.path:
    sys.path.insert(0, "/opt/trn_rl_repo")

P = 128
F = 768
N = 1024
H = 12
HD = 64
KO = 6        # 128-feature stripes (bf16 path)
KO8 = 3       # 256-feature DoubleRow stripes (fp8 path)
NT = 8        # token tiles
CH = 2        # query chunks
CW = 512      # chunk width
HP = 6        # head pairs
FW = 384      # o/v projection feature chunk
RING = 6      # psum score ring slots (1 bank each)
GRP = 3       # ring slots per ACT call (FD = 1536)
SLOTS = 192   # 12 pairs x 16 (8 kt x 2 head-halves)
GROUPS = SLOTS // GRP
LAG = 4       # attn lag behind ACT, in groups

QK_MODE = os.environ.get("QK_MODE", "fp8qk")  # fp8qk | fp8k | bf16
SX = 2.0      # x pre-scale for fp8
SW = 16.0     # Wq/Wk pre-scale for fp8
SCALE = 1.0 / float(np.sqrt(np.float32(F)))

# ch-major pair order: all of chunk 0's head pairs, then chunk 1's.
PAIRS = [(ch, hp) for ch in range(CH) for hp in range(HP)]

_CACHE = {}
LAST_EXEC_NS = None


def _build():
    import concourse.mybir as mybir
    import concourse.tile as tile
    from concourse import bacc

    f32 = mybir.dt.float32
    bf16 = mybir.dt.bfloat16
    fp8 = mybir.dt.float8e4
    ADD = mybir.AluOpType.add
    SIG = mybir.ActivationFunctionType.Sigmoid
    DR = mybir.MatmulPerfMode.DoubleRow

    q_fp8 = QK_MODE == "fp8qk"
    k_fp8 = QK_MODE in ("fp8qk", "fp8k")
    any_fp8 = q_fp8 or k_fp8
    cq = SX * SW if q_fp8 else 1.0
    ck = SX * SW if k_fp8 else 1.0
    scale_act = SCALE / (cq * ck)

    nc = bacc.Bacc("TRN2", target_bir_lowering=False, debug=False)

    xT_d = nc.dram_tensor("xT", [P, KO, N], bf16, kind="ExternalInput").ap()
    if any_fp8:
        x8_d = nc.dram_tensor("x8", [P, KO, N], fp8, kind="ExternalInput").ap()
    if q_fp8:
        wq_d = nc.dram_tensor("wqT", [P, KO, F], fp8, kind="ExternalInput").ap()
    else:
        wq_d = nc.dram_tensor("wqT", [P, KO, F], bf16, kind="ExternalInput").ap()
    if k_fp8:
        wk_d = nc.dram_tensor("wkT", [P, KO, F], fp8, kind="ExternalInput").ap()
    else:
        wk_d = nc.dram_tensor("wkT", [P, KO, F], bf16, kind="ExternalInput").ap()
    wv_d = nc.dram_tensor("wvT", [P, KO, F], bf16, kind="ExternalInput").ap()
    wo_d = nc.dram_tensor("woT", [P, KO, F], bf16, kind="ExternalInput").ap()
    bq_d = nc.dram_tensor("bqs", [P, KO], f32, kind="ExternalInput").ap()
    bk_d = nc.dram_tensor("bks", [P, KO], f32, kind="ExternalInput").ap()
    bv_d = nc.dram_tensor("bvr", [P, F], f32, kind="ExternalInput").ap()
    bo_d = nc.dram_tensor("bor", [P, F], f32, kind="ExternalInput").ap()
    bi_d = nc.dram_tensor("bir", [P, 1], f32, kind="ExternalInput").ap()
    o_d = nc.dram_tensor("o", [N, F], f32, kind="ExternalOutput").ap()
    debug = bool(os.environ.get("KERNEL_DEBUG"))
    if debug:
        dbg_q = nc.dram_tensor("dbg_q", [P, KO, N], bf16,
                               kind="ExternalOutput").ap()
        dbg_k = nc.dram_tensor("dbg_k", [P, KO, N], bf16,
                               kind="ExternalOutput").ap()
        dbg_v = nc.dram_tensor("dbg_v", [P, NT, F], bf16,
                               kind="ExternalOutput").ap()
        dbg_a = nc.dram_tensor("dbg_a", [P, KO, N], bf16,
                               kind="ExternalOutput").ap()
        dbg_sc = nc.dram_tensor("dbg_sc", [P, GRP, CW], f32,
                                kind="ExternalOutput").ap()
        dbg_st = nc.dram_tensor("dbg_st", [P, GRP, CW], bf16,
                                kind="ExternalOutput").ap()

    with tile.TileContext(nc) as tc:
        with (
            tc.tile_pool(name="sb", bufs=1) as sb,
            tc.tile_pool(name="ps", bufs=1, space="PSUM") as psp,
        ):
            # ---- persistent SBUF -------------------------------------
            xT = sb.tile([P, KO, N], bf16, tag="xT")
            if any_fp8:
                x8 = sb.tile([P, KO, N], fp8, tag="x8")
            wq = sb.tile([P, KO, F], fp8 if q_fp8 else bf16, tag="wq")
            wk = sb.tile([P, KO, F], fp8 if k_fp8 else bf16, tag="wk")
            wv = sb.tile([P, KO, F], bf16, tag="wv")
            wo = sb.tile([P, KO, F], bf16, tag="wo")
            qT = sb.tile([P, KO, N], bf16, tag="qT")
            kT = sb.tile([P, KO, N], bf16, tag="kT")
            v_sb = sb.tile([P, NT, F], bf16, tag="v")
            attnT = sb.tile([P, KO, N], bf16, tag="attnT")
            o_part = sb.tile([P, 4, F], f32, tag="opart")
            bqs = sb.tile([P, KO], f32, tag="bqs")
            bks = sb.tile([P, KO], f32, tag="bks")
            bvr = sb.tile([P, F], f32, tag="bvr")
            bor = sb.tile([P, F], f32, tag="bor")
            bir = sb.tile([P, 1], f32, tag="bir")
            warm_w = sb.tile([P, P], bf16, tag="warmw")
            warm_x = sb.tile([P, 256], bf16, tag="warmx")
            dum_i = sb.tile([P, 8], f32, tag="dumi")
            dum_o = sb.tile([P, 8], bf16, tag="dumo")

            # ---- t=0 priming -----------------------------------------
            # memsets first so the PE warm-up isn't queued behind DMAs
            nc.gpsimd.memset(dum_i[:], 0.0)
            nc.gpsimd.memset(warm_w[:], 0.0)
            nc.gpsimd.memset(warm_x[:], 0.0)
            # dummy sigmoid pulls the ACT table load into the DMA wait
            nc.scalar.activation(dum_o[:], dum_i[:], SIG)
            nc.sync.dma_start(bir[:], bi_d)
            nc.sync.dma_start(bks[:], bk_d)
            nc.sync.dma_start(bqs[:], bq_d)
            if any_fp8:
                for ko in range(KO):
                    nc.sync.dma_start(x8[:, ko, :], x8_d[:, ko, :])
            else:
                for ko in range(KO):
                    nc.sync.dma_start(xT[:, ko, :], xT_d[:, ko, :])
            # stripe-0 weight columns first: unblocks the k0/q0 units
            # (and with them the first sigmoid) several us earlier
            nc.gpsimd.dma_start(wk[:, :, 0:P], wk_d[:, :, 0:P])
            nc.gpsimd.dma_start(wq[:, :, 0:P], wq_d[:, :, 0:P])
            for ko in range(KO):
                nc.gpsimd.dma_start(wk[:, ko, P:F], wk_d[:, ko, P:F])
            for ko in range(KO):
                nc.gpsimd.dma_start(wq[:, ko, P:F], wq_d[:, ko, P:F])
            # PE warm-up: a few dummy matmuls during the DMA wait
            warm_ps = psp.tile([P, CW], f32, tag="pp", bufs=1, name="pp")
            for _ in range(6):
                nc.tensor.matmul(warm_ps[:, 0:256], warm_w[:], warm_x[:],
                                 start=True, stop=True)
            if any_fp8:
                for ko in range(KO):
                    nc.sync.dma_start(xT[:, ko, :], xT_d[:, ko, :])
            for ko in range(KO):
                nc.gpsimd.dma_start(wv[:, ko, :], wv_d[:, ko, :])
            nc.sync.dma_start(bvr[:], bv_d)
            for ko in range(KO):
                nc.sync.dma_start(wo[:, ko, :], wo_d[:, ko, :])
            nc.sync.dma_start(bor[:], bo_d)

            # ---- projection / output units ---------------------------
            def unit_qk(proj, mo, ch):
                """q or k projection: stripe mo, chunk ch -> qT/kT."""
                use8 = k_fp8 if proj == "k" else q_fp8
                w_sb = wk if proj == "k" else wq
                bst = bks if proj == "k" else bqs
                dst = kT if proj == "k" else qT
                csl = slice(ch * CW, (ch + 1) * CW)
                ps = psp.tile([P, CW], f32, tag="pp", bufs=1, name="pp")
                if use8:
                    for k3 in range(KO8):
                        nc.tensor.matmul(
                            ps[:],
                            w_sb[:, 2 * k3:2 * k3 + 2, mo * P:(mo + 1) * P],
                            x8[:, 2 * k3:2 * k3 + 2, csl],
                            start=(k3 == 0), stop=(k3 == KO8 - 1),
                            perf_mode=DR,
                        )
                else:
                    for ko in range(KO):
                        nc.tensor.matmul(
                            ps[:],
                            w_sb[:, ko, mo * P:(mo + 1) * P],
                            xT[:, ko, csl],
                            start=(ko == 0), stop=(ko == KO - 1),
                        )
                nc.vector.tensor_tensor(
                    dst[:, mo, csl], ps[:],
                    bst[:, mo:mo + 1].to_broadcast([P, CW]), ADD,
                )

            def unit_v(t, j):
                """v projection: token tile t, feature chunk j."""
                ps = psp.tile([P, CW], f32, tag="pp", bufs=1, name="pp")
                psv = ps[:, 0:FW]
                for ko in range(KO):
                    nc.tensor.matmul(
                        psv,
                        xT[:, ko, t * P:(t + 1) * P],
                        wv[:, ko, j * FW:(j + 1) * FW],
                        start=(ko == 0), stop=(ko == KO - 1),
                    )
                nc.vector.tensor_tensor(
                    v_sb[:, t, j * FW:(j + 1) * FW], psv,
                    bvr[:, j * FW:(j + 1) * FW], ADD,
                )

            osb_tiles = {}

            def get_osb(tt):
                if tt not in osb_tiles:
                    osb_tiles[tt] = sb.tile([P, F], f32, tag="osb", bufs=3,
                                            name="osb")
                return osb_tiles[tt]

            def unit_o_full(tg, j):
                """chunk-0 output projection: token tile tg, half j."""
                ps = psp.tile([P, CW], f32, tag="pp", bufs=1, name="pp")
                pso = ps[:, 0:FW]
                for ko in range(KO):
                    nc.tensor.matmul(
                        pso,
                        attnT[:, ko, tg * P:(tg + 1) * P],
                        wo[:, ko, j * FW:(j + 1) * FW],
                        start=(ko == 0), stop=(ko == KO - 1),
                    )
                osb = get_osb(tg)
                nc.vector.tensor_tensor(
                    osb[:, j * FW:(j + 1) * FW], pso,
                    bor[:, j * FW:(j + 1) * FW], ADD,
                )
                if j == 1:
                    q = nc.sync if tg % 2 == 0 else nc.gpsimd
                    q.dma_start(o_d[tg * P:(tg + 1) * P, :], osb[:])
                    del osb_tiles[tg]

            def unit_o_part(tg, j):
                """chunk-1 partial o-proj (ko 0-2 + bias) -> o_part."""
                tt = 4 + tg
                ps = psp.tile([P, CW], f32, tag="pp", bufs=1, name="pp")
                pso = ps[:, 0:FW]
                for ko in range(3):
                    nc.tensor.matmul(
                        pso,
                        attnT[:, ko, tt * P:(tt + 1) * P],
                        wo[:, ko, j * FW:(j + 1) * FW],
                        start=(ko == 0), stop=(ko == 2),
                    )
                nc.vector.tensor_tensor(
                    o_part[:, tg, j * FW:(j + 1) * FW], pso,
                    bor[:, j * FW:(j + 1) * FW], ADD,
                )

            def unit_o_tail(tg):
                """chunk-1 final o-proj: ko 3-5 on freed score banks."""
                tt = 4 + tg
                ot = psp.tile([P, GRP, CW], f32, tag="sc", bufs=2, name="sc")
                osb = get_osb(tt)
                for j in range(2):
                    pso = ot[:, j, 0:FW]
                    for ko in (3, 4, 5):
                        nc.tensor.matmul(
                            pso,
                            attnT[:, ko, tt * P:(tt + 1) * P],
                            wo[:, ko, j * FW:(j + 1) * FW],
                            start=(ko == 3), stop=(ko == 5),
                        )
                    nc.vector.tensor_tensor(
                        osb[:, j * FW:(j + 1) * FW], pso,
                        o_part[:, tg, j * FW:(j + 1) * FW], ADD,
                    )
                q = nc.sync if tg % 2 == 0 else nc.gpsimd
                q.dma_start(o_d[tt * P:(tt + 1) * P, :], osb[:])
                del osb_tiles[tt]

            # ---- score / sigmoid / attention slots -------------------
            def slot_of(S):
                pair, s = divmod(S, 16)
                ch, hp = PAIRS[pair]
                kt, hd = divmod(s, 2)
                return pair, ch, hp, kt, hd

            st_tiles = {}

            def emit_score_group(g):
                sc = psp.tile([P, GRP, CW], f32, tag="sc", bufs=2, name="sc")
                for off in range(GRP):
                    S = g * GRP + off
                    pair, ch, hp, kt, hd = slot_of(S)
                    nc.tensor.matmul(
                        sc[:, off, :],
                        kT[64 * hd:64 * (hd + 1), hp, kt * P:(kt + 1) * P],
                        qT[64 * hd:64 * (hd + 1), hp, ch * CW:(ch + 1) * CW],
                        start=True, stop=True,
                    )
                return sc

            def emit_act(g, sc):
                st = sb.tile([P, GRP, CW], bf16, tag="st", bufs=10, name="st")
                nc.scalar.activation(st[:], sc[:], SIG,
                                     bias=bir[:, 0:1], scale=scale_act)
                st_tiles[g] = st

            def emit_attn_pair(pair):
                """contiguous 16-MM attention burst for one pair."""
                ch, hp = PAIRS[pair]
                at = psp.tile([P, CW], f32, tag="at", bufs=1, name="at")
                for s in range(16):
                    S = 16 * pair + s
                    kt, hd = divmod(s, 2)
                    g, off = divmod(S, GRP)
                    nc.tensor.matmul(
                        at[64 * hd:64 * (hd + 1), :],
                        v_sb[:, kt, hp * P + 64 * hd: hp * P + 64 * (hd + 1)],
                        st_tiles[g][:, off, :],
                        start=(kt == 0), stop=(kt == NT - 1),
                    )
                nc.vector.tensor_copy(attnT[:, hp, ch * CW:(ch + 1) * CW],
                                      at[:])

            # ---- filler schedule (due_group, emit_fn) ----------------
            fillers = []

            def add(due, fn, *args):
                fillers.append((due, fn, args))

            # Deadline rules: a unit must be EMITTED before anything that
            # reads its output (in-order dependency discovery), and spread
            # out so the PE FIFO never buries upcoming score MMs.
            # k_s / q0_s before score group of pair s ((16s)//3); q1_s
            # before pair 6+s; v-j0 before burst(0) at group 8; v-j1
            # before burst(3) at group 24; o_full after CAST(p5) at 34;
            # o_part after CAST(p8) at 50.
            kq_due = {1: (2, 3, 4), 2: (6, 7, 8), 3: (10, 12, 14),
                      4: (16, 17, 19), 5: (21, 22, 23)}
            for s, (d1, d2, d3) in kq_due.items():
                add(d1, unit_qk, "k", s, 0)
                add(d2, unit_qk, "k", s, 1)
                add(d3, unit_qk, "q", s, 0)
            for t in range(NT):
                add(min(t, 6), unit_v, t, 0)
            for t in range(NT):
                add(5 + 2 * t, unit_v, t, 1)
            for s in range(HP):     # q chunk-1 stripes
                add(24 + 2 * s, unit_qk, "q", s, 1)
            for u in range(8):      # chunk-0 o-proj (needs all ch0 attnT)
                add(35 + (9 * u) // 8, unit_o_full, u // 2, u % 2)
            for u in range(8):      # chunk-1 partial o-proj (ko 0-2)
                add(51 + u, unit_o_part, u // 2, u % 2)
            fillers.sort(key=lambda e: e[0])

            # ---- pre-loop: stripe-0 projections ----------------------
            unit_qk("k", 0, 0)
            unit_qk("k", 0, 1)
            unit_qk("q", 0, 0)

            if debug:
                sc_snap = sb.tile([P, GRP, CW], f32, tag="scsnap")
                st_snap = sb.tile([P, GRP, CW], bf16, tag="stsnap")

            # pair p's last score slot lives in group (16p+15)//3; delay
            # the attn burst 3 further groups so early-pair v/projection
            # fillers get more slack (st bufs=10 covers the extra lag)
            burst_at = {}
            late_pairs = []
            for p in range(len(PAIRS)):
                g = (16 * p + 15) // GRP + 3
                if g < GROUPS:
                    burst_at[g] = p
                else:
                    late_pairs.append(p)

            # ---- main slot loop --------------------------------------
            fi = 0
            for g in range(GROUPS):
                sc = emit_score_group(g)
                if debug and g == 0:
                    nc.vector.tensor_copy(sc_snap[:], sc[:])
                emit_act(g, sc)
                if debug and g == 0:
                    nc.vector.tensor_copy(st_snap[:], st_tiles[0][:])
                if g in burst_at:
                    emit_attn_pair(burst_at[g])
                while fi < len(fillers) and fillers[fi][0] <= g:
                    _, fn, args = fillers[fi]
                    fn(*args)
                    fi += 1

            # ---- drain -----------------------------------------------
            while fi < len(fillers):
                _, fn, args = fillers[fi]
                fn(*args)
                fi += 1
            for p in late_pairs:
                emit_attn_pair(p)
            for tg in range(4):
                unit_o_tail(tg)
            if debug:
                nc.sync.dma_start(dbg_q, qT[:])
                nc.sync.dma_start(dbg_k, kT[:])
                nc.sync.dma_start(dbg_v, v_sb[:])
                nc.sync.dma_start(dbg_a, attnT[:])
                nc.sync.dma_start(dbg_sc, sc_snap[:])
                nc.sync.dma_start(dbg_st, st_snap[:])

    nc.compile()
    return nc


def _bf16(a):
    import ml_dtypes
    return np.ascontiguousarray(a).astype(ml_dtypes.bfloat16)


def _fp8(a):
    import ml_dtypes
    return np.ascontiguousarray(a).astype(ml_dtypes.float8_e4m3)


def _stripe(W):
    """[F_in, F_out] -> [P, KO, F_out] feature-major stripes."""
    return np.ascontiguousarray(
        np.asarray(W, np.float32).reshape(KO, P, F).transpose(1, 0, 2))


def kernel(x, bias, Wq, bq, Wk, bk, Wv, bv, Wo, bo):
    global LAST_EXEC_NS
    from concourse import bass_utils

    q_fp8 = QK_MODE == "fp8qk"
    k_fp8 = QK_MODE in ("fp8qk", "fp8k")
    any_fp8 = q_fp8 or k_fp8
    cq = SX * SW if q_fp8 else 1.0
    ck = SX * SW if k_fp8 else 1.0

    if "nc" not in _CACHE:
        _CACHE["nc"] = _build()
    nc = _CACHE["nc"]

    x = np.asarray(x, dtype=np.float32)
    Wq = np.asarray(Wq, np.float32).T
    Wk = np.asarray(Wk, np.float32).T
    shared = {
        "wqT": _fp8(_stripe(Wq * SW)) if q_fp8 else _bf16(_stripe(Wq)),
        "wkT": _fp8(_stripe(Wk * SW)) if k_fp8 else _bf16(_stripe(Wk)),
        "wvT": _bf16(_stripe(np.asarray(Wv, np.float32).T)),
        "woT": _bf16(_stripe(np.asarray(Wo, np.float32).T)),
        "bqs": np.ascontiguousarray(
            (np.asarray(bq, np.float32) * cq).reshape(KO, P).T),
        "bks": np.ascontiguousarray(
            (np.asarray(bk, np.float32) * ck).reshape(KO, P).T),
        "bvr": np.ascontiguousarray(
            np.broadcast_to(np.asarray(bv, np.float32), (P, F))),
        "bor": np.ascontiguousarray(
            np.broadcast_to(np.asarray(bo, np.float32), (P, F))),
        "bir": np.full((P, 1), np.float32(np.asarray(bias)),
                       dtype=np.float32),
    }
    in_maps = []
    for b in range(x.shape[0]):
        m = dict(shared)
        xb = np.ascontiguousarray(x[b].T.reshape(KO, P, N).transpose(1, 0, 2))
        m["xT"] = _bf16(xb)
        if any_fp8:
            m["x8"] = _fp8(xb * SX)
        in_maps.append(m)

    trace = bool(os.environ.get("KERNEL_TRACE"))
    tdir = os.environ.get("KERNEL_TRACE_DIR") or None
    if trace:
        try:
            import ntff_hook
            ntff_hook.install()
        except Exception:
            trace = False

    res = bass_utils.run_bass_kernel_spmd(
        nc, in_maps, core_ids=list(range(len(in_maps))), trace=trace,
        tmpdir=(tdir if trace else None))
    LAST_EXEC_NS = res.exec_time_ns
    globals()["LAST_RES"] = res
    return np.stack([r["o"] for r in res.results]).astype(np.float32)


# revision 31
# speedup vs baseline: 1.0365x; 1.0191x over previous
"""Multi-head sigmoid self-attention on 8 Trainium2 NeuronCores.

Sharding: pure data parallel - batch (8) split one element per core.

Per-core design (engine-balance driven):
  * The Scalar (ACT) engine is the critical resource: 12.6M sigmoids
    ~= 82 us of pure streaming at 1 elem/cycle/lane @1.2GHz.  All
    scheduling exists to keep ACT 100% busy from ~5 us to the end.
  * Scores stream through a 6-bank PSUM ring (1 bank = one 128x512
    score tile for one head-half of one key tile).  The ACT consumes
    the ring in FD=1536 calls (3 slots each, 64 calls total) to
    amortize the ~280-cycle per-call overhead.
  * Slot-granular software pipeline: per 3-slot group the emission is
    [3 score MMs][1 ACT][3 attn MMs (lagged 4 groups)][filler units].
    Fillers (q/k/v/o projections) are deadline-scheduled so the PE
    never head-of-line blocks the score stream.
  * q/k projections run in fp8 (e4m3) with DoubleRow perf mode (2
    MACs/cell/cycle).  Host pre-scales x by SX and Wq/Wk by SW to
    center values in e4m3 range; the combined 1/(SX*SW)^2 factor is
    folded into the sigmoid's free affine scale - zero extra cost.
    v/o stay bf16 (their error reaches the output unsuppressed).
  * PE warm-up: dummy matmuls at t=0 get the HAM clock-gate to 2.4GHz
    before the first real projection; a dummy sigmoid at t=0 pulls
    the ACT table load (~2.7us) into the DMA wait.
  * Tail: the last chunk's output projection is split ko 0-2
    (precomputed into SBUF) + ko 3-5 (tail group on freed ring banks)
    so only ~5 us of work follows the final sigmoid.
"""

import os
import sys

import numpy as np

if "/opt/trn_rl_repo" not in sys.path:
    sys.path.insert(0, "/opt/trn_rl_repo")

P = 128
F = 768
N = 1024
H = 12
HD = 64
KO = 6        # 128-feature stripes (bf16 path)
KO8 = 3       # 256-feature DoubleRow stripes (fp8 path)
NT = 8        # token tiles
CH = 2        # query chunks
CW = 512      # chunk width
HP = 6        # head pairs
FW = 384      # o/v projection feature chunk
RING = 6      # psum score ring slots (1 bank each)
GRP = 3       # ring slots per ACT call (FD = 1536)
SLOTS = 192   # 12 pairs x 16 (8 kt x 2 head-halves)
GROUPS = SLOTS // GRP
LAG = 4       # attn lag behind ACT, in groups

QK_MODE = os.environ.get("QK_MODE", "fp8qk")  # fp8qk | fp8k | bf16
SX = 2.0      # x pre-scale for fp8
SW = 16.0     # Wq/Wk pre-scale for fp8
SCALE = 1.0 / float(np.sqrt(np.float32(F)))

# ch-major pair order: all of chunk 0's head pairs, then chunk 1's.
PAIRS = [(ch, hp) for ch in range(CH) for hp in range(HP)]

_CACHE = {}
LAST_EXEC_NS = None


def _build():
    import concourse.mybir as mybir
    import concourse.tile as tile
    from concourse import bacc

    f32 = mybir.dt.float32
    bf16 = mybir.dt.bfloat16
    fp8 = mybir.dt.float8e4
    ADD = mybir.AluOpType.add
    SIG = mybir.ActivationFunctionType.Sigmoid
    DR = mybir.MatmulPerfMode.DoubleRow

    q_fp8 = QK_MODE == "fp8qk"
    k_fp8 = QK_MODE in ("fp8qk", "fp8k")
    any_fp8 = q_fp8 or k_fp8
    cq = SX * SW if q_fp8 else 1.0
    ck = SX * SW if k_fp8 else 1.0
    scale_act = SCALE / (cq * ck)

    nc = bacc.Bacc("TRN2", target_bir_lowering=False, debug=False)

    xT_d = nc.dram_tensor("xT", [P, KO, N], bf16, kind="ExternalInput").ap()
    if any_fp8:
        x8_d = nc.dram_tensor("x8", [P, KO, N], fp8, kind="ExternalInput").ap()
    if q_fp8:
        wq_d = nc.dram_tensor("wqT", [P, KO, F], fp8, kind="ExternalInput").ap()
    else:
        wq_d = nc.dram_tensor("wqT", [P, KO, F], bf16, kind="ExternalInput").ap()
    if k_fp8:
        wk_d = nc.dram_tensor("wkT", [P, KO, F], fp8, kind="ExternalInput").ap()
    else:
        wk_d = nc.dram_tensor("wkT", [P, KO, F], bf16, kind="ExternalInput").ap()
    wv_d = nc.dram_tensor("wvT", [P, KO, F], bf16, kind="ExternalInput").ap()
    wo_d = nc.dram_tensor("woT", [P, KO, F], bf16, kind="ExternalInput").ap()
    bq_d = nc.dram_tensor("bqs", [P, KO], f32, kind="ExternalInput").ap()
    bk_d = nc.dram_tensor("bks", [P, KO], f32, kind="ExternalInput").ap()
    bv_d = nc.dram_tensor("bvr", [P, F], f32, kind="ExternalInput").ap()
    bo_d = nc.dram_tensor("bor", [P, F], f32, kind="ExternalInput").ap()
    bi_d = nc.dram_tensor("bir", [P, 1], f32, kind="ExternalInput").ap()
    o_d = nc.dram_tensor("o", [N, F], f32, kind="ExternalOutput").ap()
    debug = bool(os.environ.get("KERNEL_DEBUG"))
    if debug:
        dbg_q = nc.dram_tensor("dbg_q", [P, KO, N], bf16,
                               kind="ExternalOutput").ap()
        dbg_k = nc.dram_tensor("dbg_k", [P, KO, N], bf16,
                               kind="ExternalOutput").ap()
        dbg_v = nc.dram_tensor("dbg_v", [P, NT, F], bf16,
                               kind="ExternalOutput").ap()
        dbg_a = nc.dram_tensor("dbg_a", [P, KO, N], bf16,
                               kind="ExternalOutput").ap()
        dbg_sc = nc.dram_tensor("dbg_sc", [P, GRP, CW], f32,
                                kind="ExternalOutput").ap()
        dbg_st = nc.dram_tensor("dbg_st", [P, GRP, CW], bf16,
                                kind="ExternalOutput").ap()

    with tile.TileContext(nc) as tc:
        with (
            tc.tile_pool(name="sb", bufs=1) as sb,
            tc.tile_pool(name="ps", bufs=1, space="PSUM") as psp,
        ):
            # ---- persistent SBUF -------------------------------------
            # Dependency tracking for DMA'd tiles is effectively
            # tile-granular, so everything needed on the startup critical
            # path (stripe-0 weights, x) gets its own tile.
            xT = sb.tile([P, KO, N], bf16, tag="xT")
            if any_fp8:
                x8p = [sb.tile([P, 2, N], fp8, tag=f"x8p{k}", name=f"x8p{k}")
                       for k in range(KO8)]
            if q_fp8:
                wq0 = sb.tile([P, KO, P], fp8, tag="wq0")
                wqp = [sb.tile([P, 2, F], fp8, tag=f"wqp{k}", name=f"wqp{k}")
                       for k in range(KO8)]
            else:
                wq = sb.tile([P, KO, F], bf16, tag="wq")
            if k_fp8:
                wk0 = sb.tile([P, KO, P], fp8, tag="wk0")
                wkp = [sb.tile([P, 2, F], fp8, tag=f"wkp{k}", name=f"wkp{k}")
                       for k in range(KO8)]
            else:
                wk = sb.tile([P, KO, F], bf16, tag="wk")
            wv = sb.tile([P, KO, F], bf16, tag="wv")
            wo = sb.tile([P, KO, F], bf16, tag="wo")
            qT = sb.tile([P, KO, N], bf16, tag="qT")
            kT = sb.tile([P, KO, N], bf16, tag="kT")
            v_sb = sb.tile([P, NT, F], bf16, tag="v")
            attnT = sb.tile([P, KO, N], bf16, tag="attnT")
            o_part = sb.tile([P, 4, F], f32, tag="opart")
            bqs = sb.tile([P, KO], f32, tag="bqs")
            bks = sb.tile([P, KO], f32, tag="bks")
            bvr = sb.tile([P, F], f32, tag="bvr")
            bor = sb.tile([P, F], f32, tag="bor")
            bir = sb.tile([P, 1], f32, tag="bir")
            warm_w = sb.tile([P, P], bf16, tag="warmw")
            warm_x = sb.tile([P, 256], bf16, tag="warmx")
            dum_i = sb.tile([P, 8], f32, tag="dumi")
            dum_o = sb.tile([P, 8], bf16, tag="dumo")

            # ---- t=0 priming -----------------------------------------
            # memsets first so the PE warm-up isn't queued behind DMAs
            nc.gpsimd.memset(dum_i[:], 0.0)
            nc.gpsimd.memset(warm_w[:], 0.0)
            nc.gpsimd.memset(warm_x[:], 0.0)
            # dummy sigmoid pulls the ACT table load into the DMA wait
            nc.scalar.activation(dum_o[:], dum_i[:], SIG)
            # DMA priority order: startup-critical tensors first.
            # sync queue: biases, stripe-0 weights, x (fp8), x (bf16 half)
            nc.sync.dma_start(bks[:], bk_d)
            nc.sync.dma_start(bqs[:], bq_d)
            nc.sync.dma_start(bir[:], bi_d)
            if any_fp8:
                if k_fp8:
                    nc.sync.dma_start(wk0[:], wk_d[:, :, 0:P])
                if q_fp8:
                    nc.sync.dma_start(wq0[:], wq_d[:, :, 0:P])
                for k in range(KO8):
                    nc.sync.dma_start(x8p[k][:], x8_d[:, 2 * k:2 * k + 2, :])
                for ko in range(3):
                    nc.sync.dma_start(xT[:, ko, :], xT_d[:, ko, :])
            else:
                for ko in range(KO):
                    nc.sync.dma_start(xT[:, ko, :], xT_d[:, ko, :])
            # gpsimd queue: v weights, rest of x, remaining q/k weights
            for ko in range(KO):
                nc.gpsimd.dma_start(wv[:, ko, :], wv_d[:, ko, :])
            nc.gpsimd.dma_start(bvr[:], bv_d)
            if any_fp8:
                for ko in range(3, KO):
                    nc.gpsimd.dma_start(xT[:, ko, :], xT_d[:, ko, :])
            for k in range(KO8):
                if k_fp8:
                    nc.gpsimd.dma_start(wkp[k][:, :, P:F],
                                        wk_d[:, 2 * k:2 * k + 2, P:F])
                if q_fp8:
                    nc.gpsimd.dma_start(wqp[k][:, :, P:F],
                                        wq_d[:, 2 * k:2 * k + 2, P:F])
            if not k_fp8:
                for ko in range(KO):
                    nc.gpsimd.dma_start(wk[:, ko, :], wk_d[:, ko, :])
            if not q_fp8:
                for ko in range(KO):
                    nc.gpsimd.dma_start(wq[:, ko, :], wq_d[:, ko, :])
            for ko in range(KO):
                nc.gpsimd.dma_start(wo[:, ko, :], wo_d[:, ko, :])
            nc.gpsimd.dma_start(bor[:], bo_d)
            # PE warm-up: a few dummy matmuls during the DMA wait
            warm_ps = psp.tile([P, CW], f32, tag="pp", bufs=1, name="pp")
            for _ in range(6):
                nc.tensor.matmul(warm_ps[:, 0:256], warm_w[:], warm_x[:],
                                 start=True, stop=True)

            # ---- projection / output units ---------------------------
            def unit_qk(proj, mo, ch):
                """q or k projection: stripe mo, chunk ch -> qT/kT."""
                use8 = k_fp8 if proj == "k" else q_fp8
                bst = bks if proj == "k" else bqs
                dst = kT if proj == "k" else qT
                csl = slice(ch * CW, (ch + 1) * CW)
                ps = psp.tile([P, CW], f32, tag="pp", bufs=1, name="pp")
                if use8:
                    w0 = wk0 if proj == "k" else wq0
                    wp = wkp if proj == "k" else wqp
                    for k3 in range(KO8):
                        w_ap = (w0[:, 2 * k3:2 * k3 + 2, :] if mo == 0
                                else wp[k3][:, :, mo * P:(mo + 1) * P])
                        nc.tensor.matmul(
                            ps[:], w_ap, x8p[k3][:, :, csl],
                            start=(k3 == 0), stop=(k3 == KO8 - 1),
                            perf_mode=DR,
                        )
                else:
                    w_sb = wk if proj == "k" else wq
                    for ko in range(KO):
                        nc.tensor.matmul(
                            ps[:],
                            w_sb[:, ko, mo * P:(mo + 1) * P],
                            xT[:, ko, csl],
                            start=(ko == 0), stop=(ko == KO - 1),
                        )
                nc.vector.tensor_tensor(
                    dst[:, mo, csl], ps[:],
                    bst[:, mo:mo + 1].to_broadcast([P, CW]), ADD,
                )

            def unit_v(t, j):
                """v projection: token tile t, feature chunk j."""
                ps = psp.tile([P, CW], f32, tag="pp", bufs=1, name="pp")
                psv = ps[:, 0:FW]
                for ko in range(KO):
                    nc.tensor.matmul(
                        psv,
                        xT[:, ko, t * P:(t + 1) * P],
                        wv[:, ko, j * FW:(j + 1) * FW],
                        start=(ko == 0), stop=(ko == KO - 1),
                    )
                nc.vector.tensor_tensor(
                    v_sb[:, t, j * FW:(j + 1) * FW], psv,
                    bvr[:, j * FW:(j + 1) * FW], ADD,
                )

            osb_tiles = {}

            def get_osb(tt):
                if tt not in osb_tiles:
                    osb_tiles[tt] = sb.tile([P, F], f32, tag="osb", bufs=3,
                                            name="osb")
                return osb_tiles[tt]

            def unit_o_full(tg, j):
                """chunk-0 output projection: token tile tg, half j."""
                ps = psp.tile([P, CW], f32, tag="pp", bufs=1, name="pp")
                pso = ps[:, 0:FW]
                for ko in range(KO):
                    nc.tensor.matmul(
                        pso,
                        attnT[:, ko, tg * P:(tg + 1) * P],
                        wo[:, ko, j * FW:(j + 1) * FW],
                        start=(ko == 0), stop=(ko == KO - 1),
                    )
                osb = get_osb(tg)
                nc.vector.tensor_tensor(
                    osb[:, j * FW:(j + 1) * FW], pso,
                    bor[:, j * FW:(j + 1) * FW], ADD,
                )
                if j == 1:
                    q = nc.sync if tg % 2 == 0 else nc.gpsimd
                    q.dma_start(o_d[tg * P:(tg + 1) * P, :], osb[:])
                    del osb_tiles[tg]

            def unit_o_part(tg, j):
                """chunk-1 partial o-proj (ko 0-2 + bias) -> o_part.
                Alternates between the pp and at banks so the eight units
                don't serialize on one bank right before the tail."""
                tt = 4 + tg
                tag = "pp" if (tg * 2 + j) % 2 == 0 else "at"
                ps = psp.tile([P, CW], f32, tag=tag, bufs=1, name=tag)
                pso = ps[:, 0:FW]
                for ko in range(3):
                    nc.tensor.matmul(
                        pso,
                        attnT[:, ko, tt * P:(tt + 1) * P],
                        wo[:, ko, j * FW:(j + 1) * FW],
                        start=(ko == 0), stop=(ko == 2),
                    )
                nc.vector.tensor_tensor(
                    o_part[:, tg, j * FW:(j + 1) * FW], pso,
                    bor[:, j * FW:(j + 1) * FW], ADD,
                )

            def unit_o_tail(tg):
                """chunk-1 final o-proj: ko 3-5 on freed score banks.
                ko 5 (the last pair's attnT) comes from attn_last[tg]."""
                tt = 4 + tg
                ot = psp.tile([P, GRP, CW], f32, tag="sc", bufs=2, name="sc")
                osb = get_osb(tt)
                for j in range(2):
                    pso = ot[:, j, 0:FW]
                    for ko in (3, 4):
                        nc.tensor.matmul(
                            pso,
                            attnT[:, ko, tt * P:(tt + 1) * P],
                            wo[:, ko, j * FW:(j + 1) * FW],
                            start=(ko == 3), stop=False,
                        )
                    nc.tensor.matmul(
                        pso,
                        attn_last[tg][:],
                        wo[:, 5, j * FW:(j + 1) * FW],
                        start=False, stop=True,
                    )
                    nc.vector.tensor_tensor(
                        osb[:, j * FW:(j + 1) * FW], pso,
                        o_part[:, tg, j * FW:(j + 1) * FW], ADD,
                    )
                q = nc.sync if tg % 2 == 0 else nc.gpsimd
                q.dma_start(o_d[tt * P:(tt + 1) * P, :], osb[:])
                del osb_tiles[tt]

            # ---- score / sigmoid / attention slots -------------------
            def slot_of(S):
                pair, s = divmod(S, 16)
                ch, hp = PAIRS[pair]
                kt, hd = divmod(s, 2)
                return pair, ch, hp, kt, hd

            st_tiles = {}

            def emit_score_group(g):
                sc = psp.tile([P, GRP, CW], f32, tag="sc", bufs=2, name="sc")
                for off in range(GRP):
                    S = g * GRP + off
                    pair, ch, hp, kt, hd = slot_of(S)
                    nc.tensor.matmul(
                        sc[:, off, :],
                        kT[64 * hd:64 * (hd + 1), hp, kt * P:(kt + 1) * P],
                        qT[64 * hd:64 * (hd + 1), hp, ch * CW:(ch + 1) * CW],
                        start=True, stop=True,
                    )
                return sc

            def emit_act(g, sc):
                st = sb.tile([P, GRP, CW], bf16, tag="st", bufs=10, name="st")
                nc.scalar.activation(st[:], sc[:], SIG,
                                     bias=bir[:, 0:1], scale=scale_act)
                st_tiles[g] = st

            # last pair's attnT lands in per-token-tile tiles so each
            # tail o-proj unit waits only on its own slice's copy-out
            attn_last = [sb.tile([P, P], bf16, tag=f"alast{i}", name=f"alast{i}")
                         for i in range(4)]

            def emit_attn_pair(pair, split_cast=False):
                """contiguous 16-MM attention burst for one pair."""
                ch, hp = PAIRS[pair]
                at = psp.tile([P, CW], f32, tag="at", bufs=1, name="at")
                for s in range(16):
                    S = 16 * pair + s
                    kt, hd = divmod(s, 2)
                    g, off = divmod(S, GRP)
                    nc.tensor.matmul(
                        at[64 * hd:64 * (hd + 1), :],
                        v_sb[:, kt, hp * P + 64 * hd: hp * P + 64 * (hd + 1)],
                        st_tiles[g][:, off, :],
                        start=(kt == 0), stop=(kt == NT - 1),
                    )
                if split_cast:
                    for i in range(4):
                        nc.vector.tensor_copy(attn_last[i][:],
                                              at[:, i * P:(i + 1) * P])
                else:
                    nc.vector.tensor_copy(attnT[:, hp, ch * CW:(ch + 1) * CW],
                                          at[:])

            # ---- filler schedule (due_group, emit_fn) ----------------
            fillers = []

            def add(due, fn, *args):
                fillers.append((due, fn, args))

            # Deadline rules: a unit must be EMITTED before anything that
            # reads its output (in-order dependency discovery), and spread
            # out so the PE FIFO never buries upcoming score MMs.
            # k_s / q0_s before score group of pair s ((16s)//3); q1_s
            # before pair 6+s; v-j0 before burst(0) at group 8; v-j1
            # before burst(3) at group 24; o_full after CAST(p5) at 34;
            # o_part after CAST(p8) at 50.
            kq_due = {1: (2, 3, 4), 2: (6, 7, 8), 3: (10, 12, 14),
                      4: (16, 17, 19), 5: (21, 22, 23)}
            for s, (d1, d2, d3) in kq_due.items():
                add(d1, unit_qk, "k", s, 0)
                add(d2, unit_qk, "k", s, 1)
                add(d3, unit_qk, "q", s, 0)
            for t in range(NT):
                add(min(t, 6), unit_v, t, 0)
            for t in range(NT):
                add(5 + 2 * t, unit_v, t, 1)
            for s in range(HP):     # q chunk-1 stripes
                add(24 + 2 * s, unit_qk, "q", s, 1)
            for u in range(8):      # chunk-0 o-proj (needs all ch0 attnT)
                add(35 + (9 * u) // 8, unit_o_full, u // 2, u % 2)
            for u in range(8):      # chunk-1 partial o-proj (ko 0-2)
                add(51 + u, unit_o_part, u // 2, u % 2)
            fillers.sort(key=lambda e: e[0])

            # ---- pre-loop: stripe-0 projections ----------------------
            unit_qk("k", 0, 0)
            unit_qk("k", 0, 1)
            unit_qk("q", 0, 0)

            if debug:
                sc_snap = sb.tile([P, GRP, CW], f32, tag="scsnap")
                st_snap = sb.tile([P, GRP, CW], bf16, tag="stsnap")

            # pair p's last score slot lives in group (16p+15)//3; delay
            # the attn burst 3 further groups so early-pair v/projection
            # fillers get more slack (st bufs=10 covers the extra lag)
            burst_at = {}
            late_pairs = []
            for p in range(len(PAIRS)):
                g = (16 * p + 15) // GRP + 3
                if g < GROUPS:
                    burst_at[g] = p
                else:
                    late_pairs.append(p)

            # ---- main slot loop --------------------------------------
            fi = 0
            for g in range(GROUPS):
                sc = emit_score_group(g)
                if debug and g == 0:
                    nc.vector.tensor_copy(sc_snap[:], sc[:])
                emit_act(g, sc)
                if debug and g == 0:
                    nc.vector.tensor_copy(st_snap[:], st_tiles[0][:])
                if g in burst_at:
                    p = burst_at[g]
                    emit_attn_pair(p, split_cast=(p == len(PAIRS) - 1))
                while fi < len(fillers) and fillers[fi][0] <= g:
                    _, fn, args = fillers[fi]
                    fn(*args)
                    fi += 1

            # ---- drain -----------------------------------------------
            while fi < len(fillers):
                _, fn, args = fillers[fi]
                fn(*args)
                fi += 1
            for p in late_pairs:
                emit_attn_pair(p, split_cast=True)
            for tg in range(4):
                unit_o_tail(tg)
            if debug:
                nc.sync.dma_start(dbg_q, qT[:])
                nc.sync.dma_start(dbg_k, kT[:])
                nc.sync.dma_start(dbg_v, v_sb[:])
                nc.sync.dma_start(dbg_a, attnT[:])
                nc.sync.dma_start(dbg_sc, sc_snap[:])
                nc.sync.dma_start(dbg_st, st_snap[:])

    nc.compile()
    return nc


def _bf16(a):
    import ml_dtypes
    return np.ascontiguousarray(a).astype(ml_dtypes.bfloat16)


def _fp8(a):
    import ml_dtypes
    return np.ascontiguousarray(a).astype(ml_dtypes.float8_e4m3)


def _stripe(W):
    """[F_in, F_out] -> [P, KO, F_out] feature-major stripes."""
    return np.ascontiguousarray(
        np.asarray(W, np.float32).reshape(KO, P, F).transpose(1, 0, 2))


def kernel(x, bias, Wq, bq, Wk, bk, Wv, bv, Wo, bo):
    global LAST_EXEC_NS
    from concourse import bass_utils

    q_fp8 = QK_MODE == "fp8qk"
    k_fp8 = QK_MODE in ("fp8qk", "fp8k")
    any_fp8 = q_fp8 or k_fp8
    cq = SX * SW if q_fp8 else 1.0
    ck = SX * SW if k_fp8 else 1.0

    if "nc" not in _CACHE:
        _CACHE["nc"] = _build()
    nc = _CACHE["nc"]

    x = np.asarray(x, dtype=np.float32)
    Wq = np.asarray(Wq, np.float32).T
    Wk = np.asarray(Wk, np.float32).T
    shared = {
        "wqT": _fp8(_stripe(Wq * SW)) if q_fp8 else _bf16(_stripe(Wq)),
        "wkT": _fp8(_stripe(Wk * SW)) if k_fp8 else _bf16(_stripe(Wk)),
        "wvT": _bf16(_stripe(np.asarray(Wv, np.float32).T)),
        "woT": _bf16(_stripe(np.asarray(Wo, np.float32).T)),
        "bqs": np.ascontiguousarray(
            (np.asarray(bq, np.float32) * cq).reshape(KO, P).T),
        "bks": np.ascontiguousarray(
            (np.asarray(bk, np.float32) * ck).reshape(KO, P).T),
        "bvr": np.ascontiguousarray(
            np.broadcast_to(np.asarray(bv, np.float32), (P, F))),
        "bor": np.ascontiguousarray(
            np.broadcast_to(np.asarray(bo, np.float32), (P, F))),
        "bir": np.full((P, 1), np.float32(np.asarray(bias)),
                       dtype=np.float32),
    }
    in_maps = []
    for b in range(x.shape[0]):
        m = dict(shared)
        xb = np.ascontiguousarray(x[b].T.reshape(KO, P, N).transpose(1, 0, 2))
        m["xT"] = _bf16(xb)
        if any_fp8:
            m["x8"] = _fp8(xb * SX)
        in_maps.append(m)

    trace = bool(os.environ.get("KERNEL_TRACE"))
    tdir = os.environ.get("KERNEL_TRACE_DIR") or None
    if trace:
        try:
            import ntff_hook
            ntff_hook.install()
        except Exception:
            trace = False

    res = bass_utils.run_bass_kernel_spmd(
        nc, in_maps, core_ids=list(range(len(in_maps))), trace=trace,
        tmpdir=(tdir if trace else None))
    LAST_EXEC_NS = res.exec_time_ns
    globals()["LAST_RES"] = res
    return np.stack([r["o"] for r in res.results]).astype(np.float32)
